# revision 1
# baseline (speedup 1.0000x reference)
"""Trainium2 Bass kernel for nn_MultiHeadAttention_91190745628911.

Full (unsharded) inputs in, full output out. Sharding: data parallel on
batch (2) x tensor parallel on heads (4 groups of 4 heads) = 8 cores.
Each core computes LN + its QKV slice + attention for its 4 heads + a
partial output projection; the host sums the 4 partials per batch and
transposes back to (seq, batch, hidden).

Self-contained: hardcodes all shapes from the problem spec.
"""
import numpy as np
import ml_dtypes
from contextlib import ExitStack

import concourse.bass as bass
import concourse.tile as tile
from concourse import bacc, mybir
from concourse.bass_utils import run_bass_kernel_spmd
from concourse.tile_rust import add_dep_helper

F32 = mybir.dt.float32
F32R = mybir.dt.float32r
BF16 = mybir.dt.bfloat16
F16 = mybir.dt.float16

SEQ, BATCH, HIDDEN = 2048, 2, 1024
NUM_HEADS, HEAD_DIM = 16, 64
N_CORES = 8
CORES_PER_BATCH = 4
HEADS_PER_CORE = NUM_HEADS // CORES_PER_BATCH  # 4
LN_EPS = 1e-6


class Cfg:
    def __init__(self, S=SEQ, E=HIDDEN, NH=HEADS_PER_CORE, HD=HEAD_DIM):
        self.S, self.E, self.NH, self.HD = S, E, NH, HD
        self.EC = E // 128              # e-chunks
        self.ST = S // 128              # s-tiles
        self.F = NH * HD                # features per core per projection
        self.FC = self.F // 128         # f-chunk (head-pair) tiles
        self.KC = S // 128              # k-chunks
        self.QHALF = min(1024, S)
        self.NQH = S // self.QHALF
        self.QB = min(512, self.QHALF)
        self.NQB = self.QHALF // self.QB
        self.SB = min(512, S)           # s-block for projections
        self.NSB = S // self.SB
        self.TRG = min(4, self.EC)      # transposes grouped per psum bank
        assert self.F % 128 == 0


def build_nc(cfg: Cfg):
    nc = bacc.Bacc("TRN2", target_bir_lowering=False, debug=False)
    S, E, NH, HD = cfg.S, cfg.E, cfg.NH, cfg.HD
    EC, ST, F, FC, KC = cfg.EC, cfg.ST, cfg.F, cfg.FC, cfg.KC
    QHALF, NQH, QB, NQB = cfg.QHALF, cfg.NQH, cfg.QB, cfg.NQB
    SB, NSB, TRG = cfg.SB, cfg.NSB, cfg.TRG

    x_d = nc.dram_tensor("x", [S, E], F32, kind="ExternalInput")
    wq_d = nc.dram_tensor("wq", [E, F], F32R, kind="ExternalInput")
    wk_d = nc.dram_tensor("wk", [E, F], F32R, kind="ExternalInput")
    wv_d = nc.dram_tensor("wv", [E, F], F32R, kind="ExternalInput")
    wo_d = nc.dram_tensor("wo", [F, E], F32R, kind="ExternalInput")
    gamma_d = nc.dram_tensor("gamma", [E], F32, kind="ExternalInput")
    beta_d = nc.dram_tensor("beta", [E], F32R, kind="ExternalInput")
    ident_d = nc.dram_tensor("ident", [128, 128], F32R, kind="ExternalInput")
    zeros_d = nc.dram_tensor("zeros", [S], F32R, kind="ExternalInput")
    maskT_d = nc.dram_tensor("maskT", [S, S], F16, kind="ExternalInput")
    out_d = nc.dram_tensor("outT", [E, S], F32, kind="ExternalOutput")
    scr_d = nc.dram_tensor("scr", [NQH * NH, QHALF], F32)   # sums bounce
    scr2_d = nc.dram_tensor("scr2", [NQH * NH, QHALF], F32)  # recip bounce

    with tile.TileContext(nc) as tc, ExitStack() as ctx:
        # ---------- persistent pools ----------
        singles = ctx.enter_context(tc.tile_pool(name="singles", bufs=1))
        big = ctx.enter_context(tc.tile_pool(name="big", bufs=1))

        ident_sb = singles.tile([128, 128], F32R)
        nc.sync.dma_start(out=ident_sb, in_=ident_d.ap())
        gamma_sb = singles.tile([128, EC], F32)
        nc.sync.dma_start(out=gamma_sb,
                          in_=gamma_d.ap().rearrange("(ec p) -> p ec", p=128))
        eps_sb = singles.tile([128, 1], F32)
        nc.vector.memset(eps_sb, LN_EPS)

        # persistent activation storages
        qTp = big.tile([128, NH, S], F32R)   # per-head, K-padded with zeros
        kT = big.tile([128, FC, S], F32R)    # head-pair packed
        v_sb = big.tile([128, KC, NH, 66], F16)

        nc.vector.memset(v_sb[:, :, :, 64:66], 1.0)

        with ExitStack() as ab_ctx:
            wpool = ab_ctx.enter_context(tc.tile_pool(name="wpool", bufs=1))
            phAB = ab_ctx.enter_context(tc.tile_pool(name="phAB", bufs=1))

            lnT = phAB.tile([128, EC, S], F32R)
            vT = phAB.tile([128, FC, S], F32R)

            # ---------- Phase A: layernorm + transpose ----------
            n_sub = E // min(512, E)
            with tc.tile_pool(name="phA", bufs=3) as phA, \
                 tc.tile_pool(name="phAst", bufs=4) as phAst, \
                 tc.tile_pool(name="psA", bufs=2, space="PSUM") as psA:
                for t in range(ST):
                    x_t = phA.tile([128, E], F32, tag="x")
                    nc.sync.dma_start(out=x_t,
                                      in_=x_d.ap()[t * 128:(t + 1) * 128, :])
                    st = phAst.tile([128, n_sub, nc.vector.BN_STATS_DIM], F32,
                                    tag="st")
                    xr = x_t.rearrange("p (a b) -> p a b", a=n_sub)
                    for i in range(n_sub):
                        nc.vector.bn_stats(out=st[:, i, :], in_=xr[:, i, :])
                    mv = phAst.tile([128, nc.vector.BN_AGGR_DIM], F32, tag="mv")
                    nc.vector.bn_aggr(out=mv, in_=st)
                    lgA = phAst.tile([128, 1], F32, tag="lgA")
                    nc.scalar.activation(lgA, mv[:, 1:2],
                                         mybir.ActivationFunctionType.Ln,
                                         bias=eps_sb[:], scale=1.0)
                    rstd = phAst.tile([128, 1], F32, tag="rstd")
                    nc.scalar.activation(rstd, lgA,
                                         mybir.ActivationFunctionType.Exp,
                                         scale=-0.5)
                    ln_t = phA.tile([128, E], F32R, tag="ln")
                    nc.vector.tensor_scalar(
                        out=ln_t, in0=x_t, scalar1=mv[:, 0:1], scalar2=rstd,
                        op0=mybir.AluOpType.subtract, op1=mybir.AluOpType.mult)
                    for g in range(EC // TRG):
                        tr = psA.tile([128, TRG, 128], F32R, tag="tr")
                        for j in range(TRG):
                            ec = g * TRG + j
                            nc.tensor.transpose(
                                tr[:, j, :], ln_t[:, ec * 128:(ec + 1) * 128],
                                ident_sb)
                        nc.vector.tensor_copy(
                            lnT[:, g * TRG:(g + 1) * TRG, t * 128:(t + 1) * 128],
                            tr)

            # weights [128, EC, F]; bias = beta @ w computed on the
            # unfolded weights, then gamma folded into w in place
            beta_cols = []
            for ec in range(EC):
                bc = singles.tile([128, 4], F32R, tag=f"betac{ec}")
                for j in range(4):
                    nc.sync.dma_start(
                        out=bc[:, j:j + 1],
                        in_=beta_d.ap()[ec * 128:(ec + 1) * 128].rearrange(
                            "(p o) -> p o", o=1))
                beta_cols.append(bc)
            # zero the unused half of each head's qTp stripe
            for h in range(NH):
                hh = h % 2
                z0 = 0 if hh == 1 else 64
                src = bass.AP(tensor=zeros_d, offset=0, ap=[[0, 64], [1, S]])
                nc.sync.dma_start(out=qTp[z0:z0 + 64, h, :], in_=src)
            w_sbs = {}
            bias_sbs = {}
            with tc.tile_pool(name="psB", bufs=2, space="PSUM") as psB:
                for name, d in (("q", wq_d), ("k", wk_d), ("v", wv_d)):
                    w_sb = wpool.tile([128, EC, F], F32R, tag=f"w{name}")
                    nc.sync.dma_start(
                        out=w_sb,
                        in_=d.ap().rearrange("(ec p) f -> p ec f", p=128))
                    w_sbs[name] = w_sb
                    b_sb = singles.tile([128, FC], F32, tag=f"bias{name}")
                    for fc in range(FC):
                        ps = psB.tile([128, 4], F32, tag="bias_ps")
                        for ec in range(EC):
                            nc.tensor.matmul(
                                ps,
                                lhsT=w_sb[:, ec, fc * 128:(fc + 1) * 128],
                                rhs=beta_cols[ec][:],
                                start=(ec == 0), stop=(ec == EC - 1))
                        nc.vector.tensor_copy(b_sb[:, fc:fc + 1], ps[:, 0:1])
                    bias_sbs[name] = b_sb
                    for ec in range(EC):
                        nc.vector.tensor_scalar_mul(
                            w_sb[:, ec, :], w_sb[:, ec, :],
                            gamma_sb[:, ec:ec + 1])

            # ---------- Phase B: QKV projections (transposed outputs) ----------
            with tc.tile_pool(name="psQKV", bufs=4, space="PSUM") as psQ:
                for name in ("q", "k", "v"):
                    w_sb, b_sb = w_sbs[name], bias_sbs[name]
                    for fc in range(FC):
                        for sb in range(NSB):
                            ps = psQ.tile([128, SB], F32, tag="qkv_ps")
                            for ec in range(EC):
                                nc.tensor.matmul(
                                    ps,
                                    lhsT=w_sb[:, ec, fc * 128:(fc + 1) * 128],
                                    rhs=lnT[:, ec, sb * SB:(sb + 1) * SB],
                                    start=(ec == 0), stop=(ec == EC - 1))
                            sl = slice(sb * SB, (sb + 1) * SB)
                            if name == "q":
                                for hh in range(2):
                                    pr = slice(hh * 64, hh * 64 + 64)
                                    nc.scalar.add(
                                        qTp[pr, 2 * fc + hh, sl],
                                        ps[pr, :], b_sb[pr, fc:fc + 1])
                            else:
                                t_sb = kT if name == "k" else vT
                                nc.scalar.add(
                                    t_sb[:, fc, sl], ps,
                                    b_sb[:, fc:fc + 1])

            # v natural layout [k-part, kc, head, 65] bf16 (65th col = ones)
            with tc.tile_pool(name="psV", bufs=2, space="PSUM") as psV:
                for fc in range(FC):
                    for kc in range(KC):
                        tr = psV.tile([128, 128], F32R, tag="vtr")
                        nc.tensor.transpose(
                            tr, vT[:, fc, kc * 128:(kc + 1) * 128], ident_sb)
                        nc.vector.tensor_copy(
                            v_sb[:, kc, fc * 2:fc * 2 + 2, 0:64],
                            tr.rearrange("p (h d) -> p h d", d=64))

        # ---------- Phase C: attention ----------
        phCD = ctx.enter_context(tc.tile_pool(name="phCD", bufs=1))
        ctxT = phCD.tile([128, FC, S], F32R)
        wo_sb = phCD.tile([128, FC, E], F32R)
        nc.sync.dma_start(out=wo_sb,
                          in_=wo_d.ap().rearrange("(fc p) e -> p fc e", p=128))
        with tc.tile_pool(name="phC", bufs=2) as phC, \
             tc.tile_pool(name="maskp", bufs=3) as maskp, \
             tc.tile_pool(name="psRing", bufs=1, space="PSUM") as psRing, \
             tc.tile_pool(name="psCtx", bufs=1, space="PSUM") as psCtx:
            KH = KC // 2 if (KC >= 8 and ((KC // 2 - 1) % 3) != 0) else KC
            for qh in range(NQH):
                mask_halves = []
                for g in range(KC // KH):
                    mh = maskp.tile([128, KH, QHALF], F16, tag="mask")
                    for j in range(KH):
                        kc = g * KH + j
                        nc.sync.dma_start(
                            out=mh[:, j, :],
                            in_=maskT_d.ap()[kc * 128:(kc + 1) * 128,
                                             qh * QHALF:(qh + 1) * QHALF])
                    mask_halves.append(mh)
                head_stash = []
                for h in range(NH):
                    hp = h // 2
                    ctx_t = psCtx.tile([128, QHALF], F32, tag="ctx")
                    ring = psRing.tile([128, 3, QHALF], F32, tag="ring")

                    def flush(kc_lo, nk, ring=ring, ctx_t=ctx_t,
                              mask_halves=mask_halves, h=h):
                        s0 = kc_lo % 3
                        at = phC.tile([128, 2, QHALF], F16, tag="attn", bufs=4)
                        nc.scalar.activation(
                            at[:, 0:nk, :], ring[:, s0:s0 + nk, :],
                            mybir.ActivationFunctionType.Exp)
                        g = kc_lo // KH
                        off = kc_lo % KH
                        assert (kc_lo + nk - 1) // KH == g
                        for j in range(nk):
                            kcj = kc_lo + j
                            nc.vector.tensor_tensor(
                                out=at[:, j:j + 1, :], in0=at[:, j:j + 1, :],
                                in1=mask_halves[g][:, off + j:off + j + 1, :],
                                op=mybir.AluOpType.mult)
                            for qb in range(NQB):
                                nc.tensor.matmul(
                                    ctx_t[0:66, qb * QB:(qb + 1) * QB],
                                    lhsT=v_sb[:, kcj, h, :],
                                    rhs=at[:, j, qb * QB:(qb + 1) * QB],
                                    start=(kcj == 0), stop=(kcj == KC - 1))

                    for kc in range(KC):
                        slot = kc % 3
                        for qb in range(NQB):
                            nc.tensor.matmul(
                                ring[:, slot, qb * QB:(qb + 1) * QB],
                                lhsT=kT[:, hp, kc * 128:(kc + 1) * 128],
                                rhs=qTp[:, h,
                                        qh * QHALF + qb * QB:
                                        qh * QHALF + (qb + 1) * QB],
                                start=True, stop=True)
                        if slot == 1:
                            flush(kc - 1, 2)
                        elif slot == 2:
                            flush(kc, 1)
                    if KC >= 3 and (KC - 1) % 3 == 0:
                        flush(KC - 1, 1)

                    # free ctx psum fast: copy out unnormalized; stash
                    stgU = phC.tile([66, QHALF], F32, tag="stgU", bufs=4)
                    nc.vector.tensor_copy(stgU, ctx_t[0:66, :])
                    r = qh * NH + h
                    sums = phC.tile([1, QHALF], F32, tag="sums")
                    nc.vector.tensor_copy(sums, stgU[64:65, :])
                    wr = nc.sync.dma_start(out=scr_d.ap()[r:r + 1, :],
                                           in_=sums[0:1, :])
                    head_stash.append((h, stgU, wr))
                if True:
                    # batched recip for this q-half (overlaps next q-half)
                    sums_all = phC.tile([NH, QHALF], F32, tag="sums_all")
                    rd0 = nc.sync.dma_start(
                        out=sums_all,
                        in_=scr_d.ap()[qh * NH:(qh + 1) * NH, :])
                    for _, _, w in head_stash:
                        add_dep_helper(rd0.ins, w.ins, reason="sums RAW")
                    lg_c = phC.tile([NH, QHALF], F32, tag="lg_c")
                    nc.scalar.activation(lg_c, sums_all,
                                         mybir.ActivationFunctionType.Ln)
                    nc.scalar.activation(lg_c, lg_c,
                                         mybir.ActivationFunctionType.Exp,
                                         scale=-1.0)
                    wr2 = nc.sync.dma_start(
                        out=scr2_d.ap()[qh * NH:(qh + 1) * NH, :],
                        in_=lg_c)
                    qsl = slice(qh * QHALF, (qh + 1) * QHALF)
                    for h, stgU, _ in head_stash:
                        hp = h // 2
                        rbc = phC.tile([64, QHALF], F32, tag="rbc")
                        src = bass.AP(tensor=scr2_d,
                                      offset=(qh * NH + h) * QHALF,
                                      ap=[[0, 64], [1, QHALF]])
                        rdh = nc.sync.dma_start(out=rbc, in_=src)
                        add_dep_helper(rdh.ins, wr2.ins, reason="recip RAW")
                        if h % 2 == 0:
                            nc.vector.scalar_tensor_tensor(
                                out=ctxT[0:64, hp, qsl],
                                in0=stgU[0:64, :], scalar=1.0, in1=rbc,
                                op0=mybir.AluOpType.mult,
                                op1=mybir.AluOpType.mult)
                        else:
                            stg = phC.tile([64, QHALF], F32R, tag="stg")
                            nc.vector.scalar_tensor_tensor(
                                out=stg, in0=stgU[0:64, :], scalar=1.0,
                                in1=rbc, op0=mybir.AluOpType.mult,
                                op1=mybir.AluOpType.mult)
                            nc.sync.dma_start(out=ctxT[64:128, hp, qsl],
                                              in_=stg)

        # ---------- Phase D: output projection (transposed) ----------
        with tc.tile_pool(name="phD", bufs=4) as phD, \
             tc.tile_pool(name="psD", bufs=6, space="PSUM") as psD:
            for ec in range(EC):
                for sb in range(NSB):
                    ps = psD.tile([128, SB], F32, tag="o_ps")
                    for fc in range(FC):
                        nc.tensor.matmul(
                            ps, lhsT=wo_sb[:, fc, ec * 128:(ec + 1) * 128],
                            rhs=ctxT[:, fc, sb * SB:(sb + 1) * SB],
                            start=(fc == 0), stop=(fc == FC - 1))
                    o_t = phD.tile([128, SB], F32, tag="o_sb")
                    nc.scalar.copy(o_t, ps)
                    nc.sync.dma_start(
                        out=out_d.ap()[ec * 128:(ec + 1) * 128,
                                       sb * SB:(sb + 1) * SB],
                        in_=o_t)

    nc.compile()
    return nc


_CACHED = {}


def _get_nc():
    if "nc" not in _CACHED:
        _CACHED["nc"] = build_nc(Cfg())
    return _CACHED["nc"]


def make_in_maps(cfg, inputs_q, mask, ln_scale, ln_bias, w_qkv, w_out,
                 n_cores=N_CORES, cores_per_batch=CORES_PER_BATCH):
    ident = np.eye(128, dtype=np.float32)
    zeros = np.zeros(cfg.S, dtype=np.float32)
    in_maps = []
    for c in range(n_cores):
        b = c // cores_per_batch
        g = c % cores_per_batch
        f0 = g * cfg.F
        f1 = f0 + cfg.F
        x_c = np.ascontiguousarray(inputs_q[:, b, :], dtype=np.float32)
        maskT_c = np.ascontiguousarray(
            (~mask[b, 0]).T).astype(np.float16)
        in_maps.append({
            "x": x_c,
            "wq": np.ascontiguousarray(w_qkv[:, 0, f0:f1], dtype=np.float32),
            "wk": np.ascontiguousarray(w_qkv[:, 1, f0:f1], dtype=np.float32),
            "wv": np.ascontiguousarray(w_qkv[:, 2, f0:f1], dtype=np.float32),
            "wo": np.ascontiguousarray(w_out[f0:f1, :], dtype=np.float32),
            "gamma": np.ascontiguousarray(ln_scale, dtype=np.float32),
            "beta": np.ascontiguousarray(ln_bias, dtype=np.float32),
            "ident": ident,
            "zeros": zeros,
            "maskT": maskT_c,
        })
    return in_maps


def combine_outputs(results):
    outTs = np.stack([results[c]["outT"] for c in range(N_CORES)])
    out = outTs.reshape(BATCH, CORES_PER_BATCH, HIDDEN, SEQ).sum(axis=1)
    return np.ascontiguousarray(out.transpose(2, 0, 1)).astype(np.float32)


def kernel(inputs_q, mask, ln_scale, ln_bias, w_qkv, w_out):
    nc = _get_nc()
    in_maps = make_in_maps(Cfg(), inputs_q, mask, ln_scale, ln_bias,
                           w_qkv, w_out)
    res = run_bass_kernel_spmd(nc, in_maps, list(range(N_CORES)))
    return combine_outputs(res.results)

